# revision 48
# baseline (speedup 1.0000x reference)
import sys

for _p in ("/opt/trn_rl_repo", "/root/.axon_site/_ro/trn_rl_repo"):
    if _p not in sys.path:
        sys.path.insert(0, _p)

import numpy as np
import ml_dtypes

from concourse import bass, bacc, mybir
from concourse.tile import TileContext
from concourse.bass_utils import run_bass_kernel_spmd

BF16 = ml_dtypes.bfloat16

# ---- problem constants (hardcoded per contract) ----
B, T, NB, D = 8, 2048, 22, 128
WIDTH = 64
FREQ = 1025
N_FFT = 2048
HOP = 512
STARTS = [0, 48, 96, 144, 192, 240, 288, 336, 384, 432, 480, 528,
          576, 624, 672, 720, 768, 816, 864, 912, 960, 961]
NCHUNK = 8          # full 128-bin frequency chunks (bins 0..1023); bin 1024 = chunk 8
TT = 512            # time-tile width for PSUM stages
PAIR = 1024         # elementwise mix stage runs on tau pairs
NPAIR = T // PAIR   # 2
OUTC = 2047         # output columns: out[512*c + r], c in [0, 2047), r in [0,512)
N_CORES = 8

def _incidence():
    inc = []
    for k in range(NCHUNK):
        lo_bin, hi_bin = 128 * k, 128 * k + 128
        bands = []
        for n, s in enumerate(STARTS):
            lo, hi = max(lo_bin, s), min(hi_bin, s + WIDTH)
            if lo < hi and not (n == 21 and lo_bin <= 1024 < hi_bin):
                bands.append(n)
        inc.append(bands)
    return inc

INC = _incidence()

def _wblock_layout():
    off = 0
    layout = {}
    for k in range(NCHUNK):
        for comp in range(2):
            for n in INC[k]:
                layout[(k, comp, n)] = off
                off += 128
    layout[(8, 0, 21)] = off
    layout[(8, 1, 21)] = off + 1
    off += 2
    return layout, off

WLAYOUT, WCOLS = _wblock_layout()

_CACHE = {}


def _build_nc():
    f32 = mybir.dt.float32
    bf16 = mybir.dt.bfloat16
    AL = mybir.AluOpType
    ACTF = mybir.ActivationFunctionType
    nc = bacc.Bacc(None, target_bir_lowering=False, debug=False)

    zp = nc.dram_tensor("zp", [128, NB, T], bf16, kind="ExternalInput")
    mixp = nc.dram_tensor("mixp", [128, NCHUNK + 1, 2, T], bf16, kind="ExternalInput")
    mpc_d = nc.dram_tensor("mpc", [128, NCHUNK + 1, FREQ], bf16, kind="ExternalInput")
    mps_d = nc.dram_tensor("mps", [128, NCHUNK, FREQ - 1], bf16, kind="ExternalInput")
    wb = nc.dram_tensor("wb", [128, WCOLS], bf16, kind="ExternalInput")
    biasb_d = nc.dram_tensor("biasb", [128, NCHUNK + 1, 2], f32, kind="ExternalInput")
    edge_d = nc.dram_tensor("edge", [128, 4, 2], f32, kind="ExternalInput")
    jrev_d = nc.dram_tensor("jrev", [128, 128], bf16, kind="ExternalInput")
    outp = nc.dram_tensor("outp", [4, 128, OUTC], f32, kind="ExternalOutput")

    def ola_range(t0, delta):
        a = t0 + delta
        fa = 0
        if a < 0:
            fa = -a
            a = 0
        b_ = t0 + delta + TT
        fb = TT
        if b_ > OUTC:
            fb = TT - (b_ - OUTC)
            b_ = OUTC
        return a, b_, fa, fb

    with TileContext(nc) as tc:
        with (
            tc.tile_pool(name="singles", bufs=1) as singles,
            tc.tile_pool(name="zpool", bufs=4) as zpool,
            tc.tile_pool(name="mixpool", bufs=6) as mixpool,
            tc.tile_pool(name="spec", bufs=33) as specpool,
            tc.tile_pool(name="mrmi", bufs=4) as mrmipool,
            tc.tile_pool(name="ppool", bufs=6) as ppool,
            tc.tile_pool(name="fpool", bufs=4) as fpool,
            tc.tile_pool(name="revbf", bufs=1) as revbfpool,
            tc.tile_pool(name="carry", bufs=8) as carrypool,
            tc.tile_pool(name="maskps", bufs=3, space="PSUM") as maskpool,
            tc.tile_pool(name="dftps", bufs=5, space="PSUM") as dftpool,
        ):
            wb_t = singles.tile([128, WCOLS], bf16, tag="wb")
            nc.sync.dma_start(wb_t[:], wb[:])
            biasb_t = singles.tile([128, NCHUNK + 1, 2], f32, tag="biasb")
            nc.sync.dma_start(biasb_t[:], biasb_d[:])
            mpc_t = singles.tile([128, NCHUNK + 1, FREQ], bf16, tag="mpc")
            mps_t = singles.tile([128, NCHUNK, FREQ - 1], bf16, tag="mps")
            edge_t = singles.tile([128, 4, 2], f32, tag="edge")
            nc.sync.dma_start(edge_t[:], edge_d[:])
            jrev_t = singles.tile([128, 128], bf16, tag="jrev")
            nc.sync.dma_start(jrev_t[:], jrev_d[:])

            wsrc = singles.tile([128, TT], bf16, tag="warm")
            nc.vector.memset(wsrc[:], 0.0)

            def pe_filler(n):
                # dependency-free matmuls into a scratch bank: keep the HAM
                # activity window busy across short PE dependency stalls
                wps = dftpool.tile([128, TT], f32, tag="dftps")
                for i in range(n):
                    nc.tensor.matmul(wps[:], wsrc[:, 0:128], wsrc[:],
                                     start=(i == 0), stop=(i == n - 1))

            pe_filler(40)

            outs, outs_rev = [], []
            for u in range(4):
                o = singles.tile([128, OUTC], f32, tag=f"out{u}")
                nc.vector.memset(o[:], 0.0)
                outs.append(o)
                orv = singles.tile([128, OUTC], f32, tag=f"outrev{u}")
                nc.gpsimd.memset(orv[:], 0.0)
                outs_rev.append(orv)
            rowacc = singles.tile([1, OUTC], f32, tag="rowacc")
            nc.vector.memset(rowacc[:], 0.0)

            def idft_ps1(t0, sr):
                ps1 = dftpool.tile([1, TT], f32, tag="dftps")
                for k in range(NCHUNK + 1):
                    kp = 128 if k < NCHUNK else 1
                    nc.tensor.matmul(ps1[:1, :], mpc_t[:kp, k, 1024:1025],
                                     sr[k][:kp, :], start=(k == 0), stop=(k == NCHUNK))
                wr = min(TT, OUTC - t0)
                nc.vector.tensor_add(rowacc[0:1, t0:t0 + wr],
                                     rowacc[0:1, t0:t0 + wr], ps1[0:1, :wr])

            def idft_blk(t0, sr, si, blk, carry_in, carry_out):
                soff = 128 * blk
                Qps = dftpool.tile([128, TT], f32, tag="dftps")
                for k in range(NCHUNK):
                    nc.tensor.matmul(Qps[:], mps_t[:, k, soff:soff + 128],
                                     si[k][:, :],
                                     start=(k == 0), stop=(k == NCHUNK - 1))
                Pps = dftpool.tile([128, TT], f32, tag="dftps")
                for k in range(NCHUNK + 1):
                    kp = 128 if k < NCHUNK else 1
                    nc.tensor.matmul(Pps[:], mpc_t[:kp, k, soff:soff + 128],
                                     sr[k][:kp, :],
                                     start=(k == 0), stop=(k == NCHUNK))
                qs2 = fpool.tile([128, TT], f32, tag="qs", bufs=2)
                nc.scalar.activation(qs2[:], Qps[:], ACTF.Identity, scale=2.0)
                # window folded into basis: P/Q pre-windowed
                fmw = fpool.tile([128, TT], f32, tag="f")
                nc.vector.scalar_tensor_tensor(fmw[:], qs2[:], -0.5, Pps[:],
                                               AL.mult, AL.add)
                # direct half: frames[s], s = 128*blk + p; col c = t + q - 2
                q, u = blk // 4, blk % 4
                a, b_, fa, fb = ola_range(t0, q - 2)
                if fb > fa:
                    o = outs[u]
                    nc.vector.tensor_add(o[:, a:b_], o[:, a:b_], fmw[:, fa:fb])
                # mirrored half into reversed-layout accumulator
                qp = 1 + (1 if blk >= 4 else 0)
                ur = 4 * qp - 1 - blk
                a, b_, fa, fb = ola_range(t0, 2 - qp)
                if blk >= 4:
                    # q'=2 ranges [t0, t0+TT) tile the columns disjointly per
                    # tau: write P+Q directly (no read-modify-write)
                    o = outs_rev[ur]
                    nc.gpsimd.tensor_add(o[:, a:b_], fmw[:, fa:fb],
                                         qs2[:, fa:fb])
                    if carry_in.get(ur) is not None:
                        ct = carry_in.pop(ur)
                        nc.gpsimd.tensor_add(o[:, a:a + 1], o[:, a:a + 1],
                                             ct[:, 0:1])
                else:
                    # q'=1 accumulates on top of this tau's q'=2 data; its
                    # last column belongs to the NEXT tau's write range, so
                    # clip it and carry that single column forward
                    fpw = fpool.tile([128, TT], f32, tag="f")
                    nc.gpsimd.tensor_add(fpw[:], fmw[:], qs2[:])
                    o = outs_rev[ur]
                    bc = min(b_, t0 + TT)
                    fc = fa + (bc - a)
                    if bc > a:
                        nc.vector.tensor_add(o[:, a:bc], o[:, a:bc],
                                             fpw[:, fa:fc])
                    if b_ > t0 + TT:
                        ct = carrypool.tile([128, 1], f32, tag="carry")
                        nc.vector.tensor_copy(ct[:, 0:1], fpw[:, TT - 1:TT])
                        carry_out[ur] = ct

            def endgame_u(u):
                # fold reversed accumulator u into outs[u] (rows 1..127)
                rb = revbfpool.tile([128, OUTC], bf16, tag="revbf")
                for c4 in range(4):
                    lo = 512 * c4
                    hi = min(lo + 512, OUTC)
                    if c4 % 2 == 0:
                        nc.scalar.copy(rb[:, lo:hi], outs_rev[u][:, lo:hi])
                    else:
                        nc.vector.tensor_copy(rb[:, lo:hi], outs_rev[u][:, lo:hi])
                for c4 in range(4):
                    lo = 512 * c4
                    hi = min(lo + 512, OUTC)
                    w = hi - lo
                    rv = maskpool.tile([128, 512], f32, tag="maskps")
                    nc.tensor.matmul(rv[:, :w], jrev_t[:], rb[:, lo:hi],
                                     start=True, stop=True)
                    nc.vector.tensor_add(outs[u][:, lo:hi], outs[u][:, lo:hi],
                                         rv[:, :w])

            def mix_chunk(k, ztiles, mtiles, sr_l, si_l):
                npart = 128 if k < NCHUNK else 1
                bands = INC[k] if k < NCHUNK else [21]
                pspair = []
                for comp in range(2):
                    ps = maskpool.tile([npart, TT], f32, tag="maskps")
                    for bi, n in enumerate(bands):
                        coloff = WLAYOUT[(k, comp, n)]
                        nc.tensor.matmul(
                            ps[:npart, :],
                            wb_t[:, coloff:coloff + npart],
                            ztiles[n][:],
                            start=(bi == 0),
                            stop=(bi == len(bands) - 1),
                        )
                    pspair.append(ps)
                ps_r, ps_i = pspair
                mr = mrmipool.tile([npart, TT], bf16, tag="mr")
                nc.scalar.activation(mr[:npart, :], ps_r[:npart, :],
                                     ACTF.Identity,
                                     bias=biasb_t[:npart, k, 0:1])
                mi = mrmipool.tile([npart, TT], bf16, tag="mi")
                nc.scalar.activation(mi[:npart, :], ps_i[:npart, :],
                                     ACTF.Identity,
                                     bias=biasb_t[:npart, k, 1:2])
                mxr = mtiles[k][:npart, 0, :]
                mxi = mtiles[k][:npart, 1, :]
                p1 = ppool.tile([npart, TT], bf16, tag="p")
                nc.vector.tensor_mul(p1[:npart, :], mr[:npart, :], mxr)
                p2 = ppool.tile([npart, TT], bf16, tag="p")
                nc.vector.tensor_mul(p2[:npart, :], mi[:npart, :], mxi)
                sr = specpool.tile([npart, TT], bf16, tag="spec")
                nc.gpsimd.tensor_sub(sr[:npart, :], p1[:npart, :], p2[:npart, :])
                sr_l.append(sr)
                if k < NCHUNK:
                    p3 = ppool.tile([npart, TT], bf16, tag="p")
                    nc.vector.tensor_mul(p3[:npart, :], mr[:npart, :], mxi)
                    p4 = ppool.tile([npart, TT], bf16, tag="p")
                    nc.vector.tensor_mul(p4[:npart, :], mi[:npart, :], mxr)
                    si = specpool.tile([npart, TT], bf16, tag="spec")
                    nc.gpsimd.tensor_add(si[:npart, :], p3[:npart, :], p4[:npart, :])
                    si_l.append(si)

            prev = None
            for tau in range(T // TT):
                t0 = tau * TT
                ztiles = []
                for n2 in range(NB // 2):
                    zt2 = zpool.tile([128, 2, TT], bf16, tag="z")
                    nc.sync.dma_start(zt2[:], zp[:, 2 * n2:2 * n2 + 2, t0:t0 + TT])
                    ztiles.append(zt2[:, 0, :])
                    ztiles.append(zt2[:, 1, :])
                mtiles = []
                for k in range(NCHUNK + 1):
                    npart = 128 if k < NCHUNK else 1
                    mt = mixpool.tile([npart, 2, TT], bf16, tag="mix")
                    nc.sync.dma_start(mt[:], mixp[:npart, k, :, t0:t0 + TT])
                    mtiles.append(mt)
                if tau == 0:
                    # basis tensors are first needed by idft(0) one stage
                    # later; issuing them after tau0's inputs unblocks the
                    # first projection sooner
                    nc.sync.dma_start(mpc_t[:], mpc_d[:])
                    nc.sync.dma_start(mps_t[:], mps_d[:])

                # interleave this tau's proj/mix chunks with the previous
                # tau's IDFT blocks so vector/gpsimd stay in step with the PE
                if tau > 0:
                    pe_filler(10)
                sr_l, si_l = [], []
                if prev is not None:
                    pt0, psr, psi = prev
                    mix_chunk(0, ztiles, mtiles, sr_l, si_l)
                    if tau == 1:
                        pe_filler(16)
                    idft_ps1(pt0, psr)
                    carry_out = {}
                    for idx in range(8):
                        mix_chunk(idx + 1, ztiles, mtiles, sr_l, si_l)
                        idft_blk(pt0, psr, psi, 7 - idx, carry_in, carry_out)
                    carry_in = carry_out
                else:
                    carry_in = {}
                    for k in range(NCHUNK + 1):
                        mix_chunk(k, ztiles, mtiles, sr_l, si_l)
                prev = (t0, sr_l, si_l)

            # drain: final tau's IDFT, endgame reversal interleaved once the
            # last mirror write to each outs_rev[u] has been issued
            pt0, psr, psi = prev
            idft_ps1(pt0, psr)
            carry_out = {}
            for blk in (4, 0, 5, 1, 6, 2, 7, 3):
                idft_blk(pt0, psr, psi, blk, carry_in, carry_out)
                if blk < 4:
                    pe_filler(6)
                    endgame_u(3 - blk)
            for u in (1, 2, 3):
                nc.vector.tensor_add(outs[u][0:1, :], outs[u][0:1, :],
                                     outs_rev[u - 1][0:1, :])
            nc.vector.tensor_add(outs[0][0:1, 1:OUTC], outs[0][0:1, 1:OUTC],
                                 outs_rev[3][0:1, 0:OUTC - 1])
            nc.vector.tensor_add(outs[0][0:1, :], outs[0][0:1, :],
                                 rowacc[0:1, :])

            # env edge fixup: columns c=0 (missing q=3 frame) and c=2046 (missing q=0)
            for u in range(4):
                for j, c in ((0, 0), (1, OUTC - 1)):
                    nc.vector.tensor_mul(outs[u][:, c:c + 1], outs[u][:, c:c + 1],
                                         edge_t[:, u, j:j + 1])
            for u in range(4):
                nc.sync.dma_start(outp[u], outs[u][:])

    if not nc.is_finalized():
        nc.finalize()
    return nc


def _host_constants():
    wgt = np.zeros(FREQ, np.float64)
    for n, s in enumerate(STARTS):
        wgt[s:s + WIDTH] += 1.0
    wgt = np.maximum(wgt, 1.0)

    # scaled IDFT basis with the Hann window folded in (win[2048-s] = win[s])
    # frames[s]*win[s] = P[s] - Q[s];  frames[2048-s]*win[2048-s] = P[s] + Q[s]
    s_idx = np.arange(N_FFT)
    win = 0.5 * (1.0 - np.cos(2.0 * np.pi * s_idx / N_FFT))
    f_idx = np.arange(FREQ)
    c_f = np.full(FREQ, 2.0)
    c_f[0] = 1.0
    c_f[N_FFT // 2] = 1.0
    sh = np.arange(FREQ)
    ang = 2.0 * np.pi * np.outer(f_idx, sh) / N_FFT
    scale = (c_f / (N_FFT * 1.5))[:, None]
    Mc = np.cos(ang) * scale * win[None, :FREQ]
    Ms = np.sin(ang) * scale * win[None, :FREQ]
    mpc = np.zeros((128, NCHUNK + 1, FREQ), np.float64)
    mps = np.zeros((128, NCHUNK, FREQ - 1), np.float64)
    for k in range(NCHUNK):
        mpc[:, k, :] = Mc[128 * k:128 * k + 128]
        mps[:, k, :] = Ms[128 * k:128 * k + 128, :FREQ - 1]
    mpc[0, 8, :] = Mc[1024]
    jrev = np.zeros((128, 128), np.float64)
    for p in range(1, 128):
        jrev[p, 128 - p] = 1.0

    w2 = win * win
    env0 = w2[np.arange(512)] + w2[512 + np.arange(512)] + w2[1024 + np.arange(512)]
    envL = w2[512 + np.arange(512)] + w2[1024 + np.arange(512)] + w2[1536 + np.arange(512)]
    edge = np.zeros((128, 4, 2), np.float32)
    for u in range(4):
        r = 128 * u + np.arange(128)
        edge[:, u, 0] = (1.5 / env0[r]).astype(np.float32)
        edge[:, u, 1] = (1.5 / envL[r]).astype(np.float32)
    return (wgt, mpc.astype(BF16), mps.astype(BF16), edge,
            jrev.astype(BF16))


def _pack_weights(W, b, wgt):
    W = np.asarray(W, np.float64)
    b = np.asarray(b, np.float64)
    W2 = np.zeros((NB, D, 128), np.float64)
    for n, s in enumerate(STARTS):
        g = wgt[s:s + WIDTH]
        W2[n, :, :WIDTH] = W[n, :, 0::2] / g[None, :]
        W2[n, :, WIDTH:] = W[n, :, 1::2] / g[None, :]
    wbp = np.zeros((128, WCOLS), np.float64)
    for key, off in WLAYOUT.items():
        k, comp, n = key
        s = STARTS[n]
        if k < NCHUNK:
            blk = np.zeros((D, 128), np.float64)
            for j in range(128):
                w = 128 * k + j - s
                if 0 <= w < WIDTH:
                    blk[:, j] = W2[n, :, comp * WIDTH + w]
            wbp[:, off:off + 128] = blk
        else:
            wbp[:, off] = W2[n, :, comp * WIDTH + 63]
    bias_f = np.zeros((FREQ, 2), np.float64)
    for f in range(FREQ):
        for n, s in enumerate(STARTS):
            w = f - s
            if 0 <= w < WIDTH:
                bias_f[f, 0] += b[n, 2 * w]
                bias_f[f, 1] += b[n, 2 * w + 1]
        bias_f[f] /= wgt[f]
    biasb = np.zeros((128, NCHUNK + 1, 2), np.float32)
    for k in range(NCHUNK):
        biasb[:, k, :] = bias_f[128 * k:128 * k + 128, :]
    biasb[0, 8, :] = bias_f[1024, :]
    return wbp.astype(BF16), biasb


def kernel(z, mix_spec, W, b):
    if "nc" not in _CACHE:
        _CACHE["nc"] = _build_nc()
        _CACHE["consts"] = _host_constants()
    nc = _CACHE["nc"]
    wgt, mpc_bf, mps_bf, edge, jrev_bf = _CACHE["consts"]
    wbp, biasb = _pack_weights(W, b, wgt)

    in_maps = []
    for core in range(N_CORES):
        zb = np.ascontiguousarray(np.transpose(z[core], (2, 1, 0))).astype(BF16)
        mixpk = np.zeros((128, NCHUNK + 1, 2, T), BF16)
        mx = mix_spec[core]  # (2, T, FREQ)
        mxT = np.transpose(mx, (0, 2, 1))  # (2, FREQ, T)
        for k in range(NCHUNK):
            mixpk[:, k, 0, :] = mxT[0, 128 * k:128 * k + 128].astype(BF16)
            mixpk[:, k, 1, :] = mxT[1, 128 * k:128 * k + 128].astype(BF16)
        mixpk[0, 8, 0, :] = mxT[0, 1024].astype(BF16)
        mixpk[0, 8, 1, :] = mxT[1, 1024].astype(BF16)
        in_maps.append({
            "zp": zb,
            "mixp": mixpk,
            "mpc": mpc_bf,
            "mps": mps_bf,
            "wb": wbp,
            "biasb": biasb,
            "edge": edge,
            "jrev": jrev_bf,
        })

    res = run_bass_kernel_spmd(nc, in_maps, core_ids=list(range(N_CORES)))
    out = np.empty((B, HOP * (T - 1)), np.float32)
    for core in range(N_CORES):
        o = res.results[core]["outp"]  # (4, 128, OUTC)
        out[core] = np.ascontiguousarray(np.transpose(o, (2, 0, 1))).reshape(-1)
    return out
